# revision 20
# baseline (speedup 1.0000x reference)
"""Bilinear pooling kernel for Trainium2 (8 NeuronCores, data-parallel over batch).

reference:
    xp = x @ W.T          [B, 2048]
    yp = y @ W.T          [B, 2048]
    z[b] = flatten(outer(xp[b], yp[b]))    [B, 2048*2048]
    out = z / max(||z||_2, 1e-12)  (row-wise L2 normalize)

Key identity: ||outer(xp, yp)||_F = ||xp||_2 * ||yp||_2, so the normalizer is
computed from xp/yp directly. The 512MB output is written once, as float16
(rel err ~5e-4, well under the 2e-2 gate), halving HBM write traffic.

The replicated weight is pre-formatted host-side as W^T in fp16 (the
sharding hint's "1024x2048 weight"), so the device does no W transpose and
reads only 4MB of weights.

Per-core plan (4 samples each):
  1. DMA x,y shards; PE-transpose into xyT16 [128, 8, 8] fp16 (i on partitions).
  2. Stream W^T fp16 in 8 k-chunks [128, 2048]; fp16 matmuls accumulate
     xp/yp into 4 psum banks [8, 512] (one per o-quarter).
  3. Norms; s_b = 1/max(||xp_b||*||yp_b||, eps); scaled fp16 yp rows ->
     DRAM bounce -> partition-broadcast DMA to ypb[b] [128, 2048].
     PE transposes proj columns into xpT2 [128, c2, jj, 8] (pair-packed:
     o = c2*256 + 2p + jj).
  4. Outer products: ot[p, jj, :] = ypb[b] * xpT2[:, c2, jj, b] (DVE fp16);
     1MB DMAs out with 8KB contiguous per partition line.
"""

import sys
from contextlib import ExitStack

import numpy as np

if "/opt/trn_rl_repo" not in sys.path:
    sys.path.insert(0, "/opt/trn_rl_repo")

B, D_IN, D_OUT = 32, 1024, 2048
NCORES = 8
BL = B // NCORES  # 4 samples per core
P = 128
KC = D_IN // P  # 8 contraction chunks
C2 = 8  # output chunks per sample: o in [c2*256, (c2+1)*256), pair-packed
EPS = 1e-12

_cache = {}


def _build_nc(debug_stop=None):
    import concourse.bass as bass  # noqa: F401
    import concourse.mybir as mybir
    import concourse.tile as tile
    from concourse import bacc

    f32 = mybir.dt.float32
    f16 = mybir.dt.float16
    nc = bacc.Bacc()

    xyt_ext = nc.declare_dram_parameter("xyt16", [P, KC, 2 * BL], f16, isOutput=False)
    wt_ext = nc.declare_dram_parameter("wt", [D_IN, D_OUT], f16, isOutput=False)
    eye8_ext = nc.declare_dram_parameter("eye8", [2 * BL, 2 * BL], f32, isOutput=False)
    mask_ext = nc.declare_dram_parameter("mask16", [2 * BL, BL, P], f16, isOutput=False)
    if debug_stop is None:
        out_ext = nc.declare_dram_parameter(
            "out", [BL, D_OUT * D_OUT], f16, isOutput=True
        )
        # out row b, flat index o*2048 + f with o = c2*256 + 2*p + jj
        out_r = out_ext[:].rearrange(
            "b (c2 p jj f) -> b c2 p jj f", c2=C2, p=P, jj=2, f=D_OUT
        )
    elif debug_stop == "proj":
        dbg_proj = nc.declare_dram_parameter(
            "dbg_proj", [2 * BL, D_OUT], f32, isOutput=True
        )
    elif debug_stop == "pre":
        dbg_xpt = nc.declare_dram_parameter(
            "dbg_xpt", [P, C2, 2, 2 * BL], f32, isOutput=True
        )
        dbg_ypb = nc.declare_dram_parameter(
            "dbg_ypb", [BL, P, D_OUT], f16, isOutput=True
        )

    # W^T row i = k*128 + p
    wt_r = wt_ext[:].rearrange("(k p) o -> k p o", k=KC, p=P)

    with tile.TileContext(nc) as tc:
        with (
            tc.tile_pool(name="const", bufs=1) as const_pool,
            tc.tile_pool(name="persist", bufs=1) as persist,
            tc.tile_pool(name="small_psum", bufs=2, space="PSUM") as small_psum,
            tc.tile_pool(name="mm_psum", bufs=1, space="PSUM") as mm_psum,
            tc.tile_pool(name="bcast_psum", bufs=2, space="PSUM") as bcast_psum,
        ):
            ident8 = const_pool.tile([2 * BL, 2 * BL], f32)
            nc.sync.dma_start(ident8[:], eye8_ext[:])
            ident1 = const_pool.tile([1, 1], f32)
            nc.vector.memset(ident1[:], 1.0)
            mask16 = const_pool.tile([2 * BL, BL, P], f16)
            nc.sync.dma_start(mask16[:], mask_ext[:])

            # ---- load xyT16 first (tiny; must not queue behind the 4MB W^T) --
            xyT16 = persist.tile([P, KC, 2 * BL], f16)
            nc.sync.dma_start(xyT16[:], xyt_ext[:])

            # ---- stream W^T k-chunks; split each across 8 DMA engines so
            # chunk k completes at ~(k+1)*1.2us instead of all-at-once ----
            wt_all = persist.tile([P, KC, D_OUT], f16)
            for k in range(KC):
                for q in range(8):
                    nc.sync.dma_start(
                        wt_all[q * 16 : (q + 1) * 16, k, :],
                        wt_r[k, q * 16 : (q + 1) * 16, :],
                    )
            # ---- projections: 4 psum banks (o-quarters), accumulate over k --
            psq = [
                mm_psum.tile([2 * BL, 512], f32, name=f"psq{oc}", tag=f"psq{oc}")
                for oc in range(4)
            ]
            for k in range(KC):
                for oc in range(4):
                    nc.tensor.matmul(
                        psq[oc][:],
                        xyT16[:, k, :],
                        wt_all[:, k, oc * 512 : (oc + 1) * 512],
                        start=(k == 0),
                        stop=(k == KC - 1),
                    )
            xy_proj = persist.tile([2 * BL, D_OUT], f32)  # rows 0-3 xp, 4-7 yp
            for oc in range(4):
                nc.vector.tensor_copy(
                    xy_proj[:, oc * 512 : (oc + 1) * 512], psq[oc][:]
                )

            # pair-packed transposes: xpT2[p, c2, jj, b] = xy_proj[b, c2*256+2p+jj]
            # c2=0,1 gate the first write tiles -> do them before the norm
            # chain; the rest go after the ypb broadcast matmuls.
            xpT2 = persist.tile([P, C2, 2, 2 * BL], f32)
            for c2 in range(2):
                pst = small_psum.tile([P, 2, 2 * BL], f32, name="pst", tag="sp")
                for jj in range(2):
                    nc.tensor.transpose(
                        pst[:, jj, :],
                        xy_proj[:, c2 * 256 + jj : (c2 + 1) * 256 : 2],
                        ident8[:],
                    )
                nc.scalar.copy(xpT2[:, c2], pst[:])

            if debug_stop == "proj":
                nc.sync.dma_start(dbg_proj[:], xy_proj[:])

            octx = ExitStack()
            if debug_stop != "proj":
                # ---- norms: ss[r] = sum_o xy_proj[r, o]^2 (ACT Square+accum
                # per o-quarter straight from psum, then tiny column add) ----
                sq_scratch = persist.tile([2 * BL, D_OUT], f32)
                ss4 = persist.tile([2 * BL, 4], f32)
                ss = persist.tile([2 * BL, 1], f32)
                for oc in range(4):
                    nc.scalar.activation(
                        sq_scratch[:, oc * 512 : (oc + 1) * 512],
                        psq[oc][:],
                        mybir.ActivationFunctionType.Square,
                        accum_out=ss4[:, oc : oc + 1],
                    )
                nc.vector.reduce_sum(ss[:], ss4[:], axis=mybir.AxisListType.X)
                ps_ss = small_psum.tile([1, 2 * BL], f32, name="ps_ss", tag="sp")
                nc.tensor.transpose(ps_ss[:], ss[:], ident8[:])
                ssT = persist.tile([1, 2 * BL], f32)
                nc.vector.tensor_copy(ssT[:], ps_ss[:])

                # s_b = 1 / max(sqrt(ssx_b * ssy_b), eps), all on partition 0
                nprod = persist.tile([1, BL], f32)
                nc.vector.tensor_tensor(
                    nprod[:], ssT[:, 0:BL], ssT[:, BL : 2 * BL], mybir.AluOpType.mult
                )
                nsqrt = persist.tile([1, BL], f32)
                nc.scalar.sqrt(nsqrt[:], nprod[:])
                nmax = persist.tile([1, BL], f32)
                nc.vector.tensor_scalar_max(nmax[:], nsqrt[:], EPS)
                sT = persist.tile([1, BL], f32)
                nc.vector.reciprocal(sT[:], nmax[:])

                # place s_b onto partition BL+b (the yp rows of xy_proj)
                sdiag = persist.tile([1, 2 * BL], f32)
                nc.vector.memset(sdiag[:], 0.0)
                nc.vector.tensor_copy(sdiag[:, BL : 2 * BL], sT[:])
                ps_sc = small_psum.tile([2 * BL, 1], f32, name="ps_sc", tag="sp")
                nc.tensor.transpose(ps_sc[:], sdiag[:], ident1[:])
                scol = persist.tile([2 * BL, 1], f32)
                nc.scalar.copy(scol[:], ps_sc[:])

                # scaled fp16 yp rows -> DRAM bounce -> partition-broadcast
                yps16 = persist.tile([P, D_OUT], f16)
                nc.vector.tensor_scalar_mul(
                    yps16[0 : 2 * BL, :],
                    xy_proj[0 : 2 * BL, :],
                    scol[0 : 2 * BL, 0:1],
                )
                ypb_pool = octx.enter_context(tc.tile_pool(name="ypb", bufs=1))
                ypb_tiles = []
                for b in range(BL):
                    ypb = ypb_pool.tile([P, D_OUT], f16, name=f"ypb{b}", tag=f"ypb{b}")
                    for n2 in range(4):
                        psb = bcast_psum.tile([P, 512], f32, name="psb", tag="psb")
                        nc.tensor.matmul(
                            psb[:],
                            mask16[:, b, :],
                            yps16[0 : 2 * BL, n2 * 512 : (n2 + 1) * 512],
                            start=True,
                            stop=True,
                        )
                        nc.scalar.copy(ypb[:, n2 * 512 : (n2 + 1) * 512], psb[:])
                    ypb_tiles.append(ypb)

                for c2 in range(2, C2):
                    pst = small_psum.tile([P, 2, 2 * BL], f32, name="pst", tag="sp")
                    for jj in range(2):
                        nc.tensor.transpose(
                            pst[:, jj, :],
                            xy_proj[:, c2 * 256 + jj : (c2 + 1) * 256 : 2],
                            ident8[:],
                        )
                    nc.scalar.copy(xpT2[:, c2], pst[:])

                if debug_stop == "pre":
                    nc.sync.dma_start(dbg_xpt[:], xpT2[:])
                    for b in range(BL):
                        nc.sync.dma_start(dbg_ypb[b], ypb_tiles[b][:])

                if debug_stop is None:
                    # ---- outer products, 1MB fp16 tiles, stream out ----
                    out_pool = octx.enter_context(tc.tile_pool(name="outp", bufs=12))
                    for b in range(BL):
                        for c2 in range(C2):
                            ot = out_pool.tile([P, 2, D_OUT], f16, name="ot")
                            for jj in range(2):
                                nc.vector.tensor_scalar_mul(
                                    ot[:, jj, :],
                                    ypb_tiles[b][:],
                                    xpT2[:, c2, jj, b : b + 1],
                                )
                            nc.sync.dma_start(out_r[b, c2], ot[:])
            octx.close()

    nc.compile()
    return nc


def _get_nc():
    if "nc" not in _cache:
        _cache["nc"] = _build_nc()
    return _cache["nc"]


def _mask16():
    m = np.zeros((2 * BL, BL, P), dtype=np.float16)
    for b in range(BL):
        m[BL + b, b, :] = 1.0
    return m


def _in_maps(x, y, W):
    x = np.asarray(x, dtype=np.float32)
    y = np.asarray(y, dtype=np.float32)
    wt16 = np.ascontiguousarray(np.asarray(W, dtype=np.float32).T, dtype=np.float16)
    eye8 = np.eye(2 * BL, dtype=np.float32)
    mask = _mask16()
    maps = []
    for c in range(NCORES):
        xy = np.concatenate(
            [x[c * BL : (c + 1) * BL], y[c * BL : (c + 1) * BL]], axis=0
        )  # [8, 1024]
        # xyT16[p, k, b] = xy[b, k*128 + p]
        xyt = np.ascontiguousarray(
            xy.T.reshape(KC, P, 2 * BL).transpose(1, 0, 2), dtype=np.float16
        )
        maps.append({"xyt16": xyt, "wt": wt16, "eye8": eye8, "mask16": mask})
    return maps


def kernel(x: np.ndarray, y: np.ndarray, W: np.ndarray) -> np.ndarray:
    from concourse.bass_utils import run_bass_kernel_spmd

    nc = _get_nc()
    res = run_bass_kernel_spmd(nc, _in_maps(x, y, W), list(range(NCORES))).results
    return np.concatenate(
        [res[c]["out"].astype(np.float32) for c in range(NCORES)], axis=0
    )


# revision 21
# speedup vs baseline: 1.3135x; 1.3135x over previous
"""Bilinear pooling kernel for Trainium2 (8 NeuronCores, data-parallel over batch).

reference:
    xp = x @ W.T          [B, 2048]
    yp = y @ W.T          [B, 2048]
    z[b] = flatten(outer(xp[b], yp[b]))    [B, 2048*2048]
    out = z / max(||z||_2, 1e-12)  (row-wise L2 normalize)

Key identity: ||outer(xp, yp)||_F = ||xp||_2 * ||yp||_2, so the normalizer is
computed from xp/yp directly. The 512MB output is written once, as float16
(rel err ~5e-4, well under the 2e-2 gate), halving HBM write traffic.

The replicated weight is pre-formatted host-side as W^T in fp16 (the
sharding hint's "1024x2048 weight"), so the device does no W transpose and
reads only 4MB of weights.

Per-core plan (4 samples each):
  1. DMA x,y shards; PE-transpose into xyT16 [128, 8, 8] fp16 (i on partitions).
  2. Stream W^T fp16 in 8 k-chunks [128, 2048]; fp16 matmuls accumulate
     xp/yp into 4 psum banks [8, 512] (one per o-quarter).
  3. Norms; s_b = 1/max(||xp_b||*||yp_b||, eps); scaled fp16 yp rows ->
     DRAM bounce -> partition-broadcast DMA to ypb[b] [128, 2048].
     PE transposes proj columns into xpT2 [128, c2, jj, 8] (pair-packed:
     o = c2*256 + 2p + jj).
  4. Outer products: ot[p, jj, :] = ypb[b] * xpT2[:, c2, jj, b] (DVE fp16);
     1MB DMAs out with 8KB contiguous per partition line.
"""

import sys
from contextlib import ExitStack

import numpy as np

if "/opt/trn_rl_repo" not in sys.path:
    sys.path.insert(0, "/opt/trn_rl_repo")

B, D_IN, D_OUT = 32, 1024, 2048
NCORES = 8
BL = B // NCORES  # 4 samples per core
P = 128
KC = D_IN // P  # 8 contraction chunks
C2 = 8  # output chunks per sample: o in [c2*256, (c2+1)*256), pair-packed
EPS = 1e-12

_cache = {}


def _build_nc(debug_stop=None):
    import concourse.bass as bass  # noqa: F401
    import concourse.mybir as mybir
    import concourse.tile as tile
    from concourse import bacc

    f32 = mybir.dt.float32
    f16 = mybir.dt.float16
    nc = bacc.Bacc()

    xyt_ext = nc.declare_dram_parameter("xyt16", [P, KC, 2 * BL], f16, isOutput=False)
    wt_ext = nc.declare_dram_parameter("wt", [D_IN, D_OUT], f16, isOutput=False)
    eye8_ext = nc.declare_dram_parameter("eye8", [2 * BL, 2 * BL], f32, isOutput=False)
    mask_ext = nc.declare_dram_parameter("mask16", [2 * BL, BL, P], f16, isOutput=False)
    if debug_stop is None:
        out_ext = nc.declare_dram_parameter(
            "out", [BL, D_OUT * D_OUT], f16, isOutput=True
        )
        # out row b, flat index o*2048 + f with o = c2*256 + 2*p + jj
        out_r = out_ext[:].rearrange(
            "b (c2 p jj f) -> b c2 p jj f", c2=C2, p=P, jj=2, f=D_OUT
        )
    elif debug_stop == "proj":
        dbg_proj = nc.declare_dram_parameter(
            "dbg_proj", [2 * BL, D_OUT], f32, isOutput=True
        )
    elif debug_stop == "pre":
        dbg_xpt = nc.declare_dram_parameter(
            "dbg_xpt", [P, C2, 2, 2 * BL], f32, isOutput=True
        )
        dbg_ypb = nc.declare_dram_parameter(
            "dbg_ypb", [BL, P, D_OUT], f16, isOutput=True
        )

    # W^T row i = k*128 + p
    wt_r = wt_ext[:].rearrange("(k p) o -> k p o", k=KC, p=P)

    with tile.TileContext(nc) as tc:
        with (
            tc.tile_pool(name="const", bufs=1) as const_pool,
            tc.tile_pool(name="persist", bufs=1) as persist,
            tc.tile_pool(name="small_psum", bufs=2, space="PSUM") as small_psum,
            tc.tile_pool(name="mm_psum", bufs=1, space="PSUM") as mm_psum,
            tc.tile_pool(name="bcast_psum", bufs=2, space="PSUM") as bcast_psum,
        ):
            ident8 = const_pool.tile([2 * BL, 2 * BL], f32)
            nc.sync.dma_start(ident8[:], eye8_ext[:])
            ident1 = const_pool.tile([1, 1], f32)
            nc.vector.memset(ident1[:], 1.0)
            mask16 = const_pool.tile([2 * BL, BL, P], f16)
            nc.sync.dma_start(mask16[:], mask_ext[:])

            # ---- load xyT16 first (tiny; must not queue behind the 4MB W^T) --
            xyT16 = persist.tile([P, KC, 2 * BL], f16)
            nc.sync.dma_start(xyT16[:], xyt_ext[:])

            # ---- stream W^T k-chunks ----
            wt_all = persist.tile([P, KC, D_OUT], f16)
            for k in range(KC):
                nc.sync.dma_start(wt_all[:, k, :], wt_r[k])
            # ---- projections: 4 psum banks (o-quarters), accumulate over k --
            psq = [
                mm_psum.tile([2 * BL, 512], f32, name=f"psq{oc}", tag=f"psq{oc}")
                for oc in range(4)
            ]
            for k in range(KC):
                for oc in range(4):
                    nc.tensor.matmul(
                        psq[oc][:],
                        xyT16[:, k, :],
                        wt_all[:, k, oc * 512 : (oc + 1) * 512],
                        start=(k == 0),
                        stop=(k == KC - 1),
                    )
            xy_proj = persist.tile([2 * BL, D_OUT], f32)  # rows 0-3 xp, 4-7 yp
            for oc in range(4):
                nc.vector.tensor_copy(
                    xy_proj[:, oc * 512 : (oc + 1) * 512], psq[oc][:]
                )

            # pair-packed transposes: xpT2[p, c2, jj, b] = xy_proj[b, c2*256+2p+jj]
            # c2=0,1 gate the first write tiles -> do them before the norm
            # chain; the rest go after the ypb broadcast matmuls.
            xpT2 = persist.tile([P, C2, 2, 2 * BL], f32)
            for c2 in range(2):
                pst = small_psum.tile([P, 2, 2 * BL], f32, name="pst", tag="sp")
                for jj in range(2):
                    nc.tensor.transpose(
                        pst[:, jj, :],
                        xy_proj[:, c2 * 256 + jj : (c2 + 1) * 256 : 2],
                        ident8[:],
                    )
                nc.scalar.copy(xpT2[:, c2], pst[:])

            if debug_stop == "proj":
                nc.sync.dma_start(dbg_proj[:], xy_proj[:])

            octx = ExitStack()
            if debug_stop != "proj":
                # ---- norms: ss[r] = sum_o xy_proj[r, o]^2 (ACT Square+accum
                # per o-quarter straight from psum, then tiny column add) ----
                sq_scratch = persist.tile([2 * BL, D_OUT], f32)
                ss4 = persist.tile([2 * BL, 4], f32)
                ss = persist.tile([2 * BL, 1], f32)
                for oc in range(4):
                    nc.scalar.activation(
                        sq_scratch[:, oc * 512 : (oc + 1) * 512],
                        psq[oc][:],
                        mybir.ActivationFunctionType.Square,
                        accum_out=ss4[:, oc : oc + 1],
                    )
                nc.vector.reduce_sum(ss[:], ss4[:], axis=mybir.AxisListType.X)
                ps_ss = small_psum.tile([1, 2 * BL], f32, name="ps_ss", tag="sp")
                nc.tensor.transpose(ps_ss[:], ss[:], ident8[:])
                ssT = persist.tile([1, 2 * BL], f32)
                nc.vector.tensor_copy(ssT[:], ps_ss[:])

                # s_b = 1 / max(sqrt(ssx_b * ssy_b), eps), all on partition 0
                nprod = persist.tile([1, BL], f32)
                nc.vector.tensor_tensor(
                    nprod[:], ssT[:, 0:BL], ssT[:, BL : 2 * BL], mybir.AluOpType.mult
                )
                nsqrt = persist.tile([1, BL], f32)
                nc.scalar.sqrt(nsqrt[:], nprod[:])
                nmax = persist.tile([1, BL], f32)
                nc.vector.tensor_scalar_max(nmax[:], nsqrt[:], EPS)
                sT = persist.tile([1, BL], f32)
                nc.vector.reciprocal(sT[:], nmax[:])

                # place s_b onto partition BL+b (the yp rows of xy_proj)
                sdiag = persist.tile([1, 2 * BL], f32)
                nc.vector.memset(sdiag[:], 0.0)
                nc.vector.tensor_copy(sdiag[:, BL : 2 * BL], sT[:])
                ps_sc = small_psum.tile([2 * BL, 1], f32, name="ps_sc", tag="sp")
                nc.tensor.transpose(ps_sc[:], sdiag[:], ident1[:])
                scol = persist.tile([2 * BL, 1], f32)
                nc.scalar.copy(scol[:], ps_sc[:])

                # scaled fp16 yp rows -> DRAM bounce -> partition-broadcast
                yps16 = persist.tile([P, D_OUT], f16)
                nc.vector.tensor_scalar_mul(
                    yps16[0 : 2 * BL, :],
                    xy_proj[0 : 2 * BL, :],
                    scol[0 : 2 * BL, 0:1],
                )
                ypb_pool = octx.enter_context(tc.tile_pool(name="ypb", bufs=1))
                ypb_tiles = []
                for b in range(BL):
                    ypb = ypb_pool.tile([P, D_OUT], f16, name=f"ypb{b}", tag=f"ypb{b}")
                    for n2 in range(4):
                        psb = bcast_psum.tile([P, 512], f32, name="psb", tag="psb")
                        nc.tensor.matmul(
                            psb[:],
                            mask16[:, b, :],
                            yps16[0 : 2 * BL, n2 * 512 : (n2 + 1) * 512],
                            start=True,
                            stop=True,
                        )
                        nc.scalar.copy(ypb[:, n2 * 512 : (n2 + 1) * 512], psb[:])
                    ypb_tiles.append(ypb)

                for c2 in range(2, C2):
                    pst = small_psum.tile([P, 2, 2 * BL], f32, name="pst", tag="sp")
                    for jj in range(2):
                        nc.tensor.transpose(
                            pst[:, jj, :],
                            xy_proj[:, c2 * 256 + jj : (c2 + 1) * 256 : 2],
                            ident8[:],
                        )
                    nc.scalar.copy(xpT2[:, c2], pst[:])

                if debug_stop == "pre":
                    nc.sync.dma_start(dbg_xpt[:], xpT2[:])
                    for b in range(BL):
                        nc.sync.dma_start(dbg_ypb[b], ypb_tiles[b][:])

                if debug_stop is None:
                    # ---- outer products, 1MB fp16 tiles, stream out ----
                    out_pool = octx.enter_context(tc.tile_pool(name="outp", bufs=12))
                    for b in range(BL):
                        for c2 in range(C2):
                            ot = out_pool.tile([P, 2, D_OUT], f16, name="ot")
                            for jj in range(2):
                                nc.vector.tensor_scalar_mul(
                                    ot[:, jj, :],
                                    ypb_tiles[b][:],
                                    xpT2[:, c2, jj, b : b + 1],
                                )
                            nc.sync.dma_start(out_r[b, c2], ot[:])
            octx.close()

    nc.compile()
    return nc


def _get_nc():
    if "nc" not in _cache:
        _cache["nc"] = _build_nc()
    return _cache["nc"]


def _mask16():
    m = np.zeros((2 * BL, BL, P), dtype=np.float16)
    for b in range(BL):
        m[BL + b, b, :] = 1.0
    return m


def _in_maps(x, y, W):
    x = np.asarray(x, dtype=np.float32)
    y = np.asarray(y, dtype=np.float32)
    wt16 = np.ascontiguousarray(np.asarray(W, dtype=np.float32).T, dtype=np.float16)
    eye8 = np.eye(2 * BL, dtype=np.float32)
    mask = _mask16()
    maps = []
    for c in range(NCORES):
        xy = np.concatenate(
            [x[c * BL : (c + 1) * BL], y[c * BL : (c + 1) * BL]], axis=0
        )  # [8, 1024]
        # xyT16[p, k, b] = xy[b, k*128 + p]
        xyt = np.ascontiguousarray(
            xy.T.reshape(KC, P, 2 * BL).transpose(1, 0, 2), dtype=np.float16
        )
        maps.append({"xyt16": xyt, "wt": wt16, "eye8": eye8, "mask16": mask})
    return maps


def kernel(x: np.ndarray, y: np.ndarray, W: np.ndarray) -> np.ndarray:
    from concourse.bass_utils import run_bass_kernel_spmd

    nc = _get_nc()
    res = run_bass_kernel_spmd(nc, _in_maps(x, y, W), list(range(NCORES))).results
    return np.concatenate(
        [res[c]["out"].astype(np.float32) for c in range(NCORES)], axis=0
    )
